# revision 14
# baseline (speedup 1.0000x reference)
"""Sliding-window GQA causal self-attention block for 8 trn2 NeuronCores.

Sharding: batch (4) x T-halves (2) -> 8 cores, no collectives. Each core gets
x.T for its T-half plus a 256-row key/value halo and computes its (1024, 1024)
slice of the output.

v3 design notes (cost-model driven):
- qkv projection runs as fp8e4 hi+lo DoubleRow matmuls (x and w split into
  fp8 hi/lo pairs on the host; 3 DR terms replace 4 bf16 matmuls per
  256-channel pair -> 25% fewer PE rows at bf16-level accuracy).
- phase 1 runs in 512-token chunks with double-buffered PSUM so rope
  (ACT/DVE) overlaps the next chunk's matmuls.
- scores are j2-batched: group-major q layout lets one bf16 matmul produce
  [128 keys, 2 heads x 128 queries] (N=256), halving score instruction count.
- band masks are multiplicative 0/1 bf16 DVE multiplies on the exp'd tile
  (stride-0 broadcast over the head pair).
- att@v is "flipped": stationary = exp'd scores, moving = v (65 cols incl a
  ones column), so cost is 65 rows per key block and the softmax denominator
  lands as a per-query PSUM column -> [128, 4] reciprocal + stride-0
  broadcast normalize on DVE.
- normalized y ([q, d] layout) is transposed back to [d, q] via PE transpose
  for the row-major output projection (bf16).
- one DMA per input tensor (host pre-transposes layouts); qT/kT/out DMAs go
  through the idle Pool (gpsimd) queue.
"""

import dataclasses

import numpy as np
import ml_dtypes

import concourse.bass as bass
import concourse.mybir as mybir
import concourse.tile as tile
from concourse import bacc
from concourse.bass_utils import run_bass_kernel_spmd

F8 = ml_dtypes.float8_e4m3fn
BF = ml_dtypes.bfloat16
F32 = mybir.dt.float32
BF16 = mybir.dt.bfloat16
FP8 = mybir.dt.float8e4
DR = mybir.MatmulPerfMode.DoubleRow

B, T, C = 4, 2048, 1024
H, KV, HD = 16, 4, 64
WIN = 256
TL = T // 2            # 1024 own rows per core
TH = TL + WIN          # 1280 with halo
WS = 32.0              # host-side weight prescale for fp8 dynamic range


def _build_program():
    nc = bacc.Bacc("TRN2", target_bir_lowering=False, debug=False, num_devices=8)
    dt = mybir.dt
    xh = nc.dram_tensor("xh", [128, 8, TH], dt.float8e4, kind="ExternalInput").ap()
    xl = nc.dram_tensor("xl", [128, 8, TH], dt.float8e4, kind="ExternalInput").ap()
    wqkh = nc.dram_tensor("wqkh", [128, 8, 1280], dt.float8e4, kind="ExternalInput").ap()
    wqkl = nc.dram_tensor("wqkl", [128, 8, 1280], dt.float8e4, kind="ExternalInput").ap()
    wvh = nc.dram_tensor("wvh", [128, 8, 256], dt.float8e4, kind="ExternalInput").ap()
    wvl = nc.dram_tensor("wvl", [128, 8, 256], dt.float8e4, kind="ExternalInput").ap()
    wp = nc.dram_tensor("wp", [128, 8, C], dt.bfloat16, kind="ExternalInput").ap()
    cq = nc.dram_tensor("cq", [128, 2, TL], dt.bfloat16, kind="ExternalInput").ap()
    ck = nc.dram_tensor("ck", [128, 2, TH], dt.bfloat16, kind="ExternalInput").ap()
    mbm = nc.dram_tensor("mbm", [128, 4, 128], dt.bfloat16, kind="ExternalInput").ap()
    idt = nc.dram_tensor("idt", [128, 128], dt.bfloat16, kind="ExternalInput").ap()
    out = nc.dram_tensor("out", [TL, C], dt.bfloat16, kind="ExternalOutput").ap()

    with tile.TileContext(nc) as tc:
        _kernel_body(tc, nc, xh, xl, wqkh, wqkl, wvh, wvl, wp, cq, ck,
                     mbm, idt, out)
    nc.compile()
    return nc


def _kernel_body(tc, nc, xh, xl, wqkh, wqkl, wvh, wvl, wp, cq, ck,
                 mbm, idt, out, dbg=None):
    import contextlib
    ctx = contextlib.ExitStack()
    with ctx:
        consts = ctx.enter_context(tc.tile_pool(name="consts", bufs=1))
        persist = ctx.enter_context(tc.tile_pool(name="persist", bufs=1))

        # ---- load persistent inputs: one DMA per tensor, spread queues ----
        xh_sb = persist.tile([128, 8, TH], FP8, tag="xh")
        xl_sb = persist.tile([128, 8, TH], FP8, tag="xl")
        wvh_sb = persist.tile([128, 8, 256], FP8, tag="wvh")
        wvl_sb = persist.tile([128, 8, 256], FP8, tag="wvl")
        wqh_sb = persist.tile([128, 8, 1280], FP8, tag="wqh")
        wql_sb = persist.tile([128, 8, 1280], FP8, tag="wql")
        wp_sb = persist.tile([128, 8, C], BF16, tag="wp")
        cq_sb = consts.tile([128, 2, TL], BF16)
        ck_sb = consts.tile([128, 2, TH], BF16)
        mb_sb = consts.tile([128, 4, 128], BF16)
        idt_sb = consts.tile([128, 128], BF16)
        nc.sync.dma_start(out=xh_sb[:], in_=xh)
        nc.sync.dma_start(out=xl_sb[:], in_=xl)
        nc.scalar.dma_start(out=wvh_sb[:], in_=wvh)
        nc.scalar.dma_start(out=wvl_sb[:], in_=wvl)
        nc.sync.dma_start(out=wqh_sb[:], in_=wqkh)
        nc.sync.dma_start(out=wql_sb[:], in_=wqkl)
        nc.scalar.dma_start(out=wp_sb[:], in_=wp)
        nc.scalar.dma_start(out=cq_sb[:], in_=cq)
        nc.scalar.dma_start(out=ck_sb[:], in_=ck)
        nc.gpsimd.dma_start(out=mb_sb[:], in_=mbm)
        nc.gpsimd.dma_start(out=idt_sb[:], in_=idt)

        # persistent compute tensors
        qTg = [persist.tile([64, 4, TL], BF16, tag=f"qTg{g}", name=f"qTg{g}")
               for g in range(KV)]
        kT = [persist.tile([64, TH], BF16, tag=f"kT{g}", name=f"kT{g}")
              for g in range(KV)]
        v65 = persist.tile([128, 10, KV, 65], BF16, tag="v65")
        yT = persist.tile([128, 8, TL], BF16, tag="yT")

        def dr3(out_ap, p, stat_h, stat_l, stat_cols, mov_h, mov_l, mov_cols,
                first, last):
            """Three hi/lo DoubleRow terms for chunk pair p (contraction
            channels [256p, 256p+256))."""
            sh = stat_h[:, 2 * p:2 * p + 2, stat_cols[0]:stat_cols[1]]
            sl = stat_l[:, 2 * p:2 * p + 2, stat_cols[0]:stat_cols[1]]
            mh = mov_h[:, 2 * p:2 * p + 2, mov_cols[0]:mov_cols[1]]
            ml = mov_l[:, 2 * p:2 * p + 2, mov_cols[0]:mov_cols[1]]
            nc.tensor.matmul(out_ap, sh, mh, start=first, stop=False,
                             perf_mode=DR)
            nc.tensor.matmul(out_ap, sh, ml, start=False, stop=False,
                             perf_mode=DR)
            nc.tensor.matmul(out_ap, sl, mh, start=False, stop=last,
                             perf_mode=DR)

        # ======== phase 1: qkv projection + rope (512-token chunks) ========
        with tc.tile_pool(name="pps", bufs=2, space="PSUM") as pps, \
             tc.tile_pool(name="vps", bufs=2, space="PSUM") as vps, \
             tc.tile_pool(name="ropes", bufs=2) as ropes:

            # v first: needs only x + wv
            for tcn in range(10):
                pv = vps.tile([128, 256], F32, tag="pv")
                tc_cols = (tcn * 128, (tcn + 1) * 128)
                for p in range(4):
                    dr3(pv[:], p, xh_sb, xl_sb, tc_cols,
                        wvh_sb, wvl_sb, (0, 256), p == 0, p == 3)
                v4 = v65[:, tcn, :, 0:64]
                nc.scalar.mul(v4, pv[:].rearrange("p (g c) -> p g c", c=64),
                              1.0 / WS)
            nc.vector.memset(v65[:, :, :, 64:65], 1.0)

            def rope_chunk(pe, po, cs_sb, c0, clen):
                e_sb = ropes.tile([128, 512], BF16, tag="e_sb")
                o_sb = ropes.tile([128, 512], BF16, tag="o_sb")
                nc.scalar.mul(e_sb[:, 0:clen], pe[:, 0:clen], 1.0 / WS)
                nc.scalar.mul(o_sb[:, 0:clen], po[:, 0:clen], 1.0 / WS)
                ne = ropes.tile([128, 512], BF16, tag="r0")
                no_ = ropes.tile([128, 512], BF16, tag="r1")
                t1 = ropes.tile([128, 512], BF16, tag="r2")
                t2 = ropes.tile([128, 512], BF16, tag="r3")
                cs = cs_sb[:, :, c0:c0 + clen]
                nc.vector.tensor_mul(t1[:, 0:clen], e_sb[:, 0:clen], cs[:, 0, :])
                nc.vector.tensor_mul(t2[:, 0:clen], o_sb[:, 0:clen], cs[:, 1, :])
                nc.vector.tensor_sub(ne[:, 0:clen], t1[:, 0:clen], t2[:, 0:clen])
                nc.vector.tensor_mul(t1[:, 0:clen], e_sb[:, 0:clen], cs[:, 1, :])
                nc.vector.tensor_mul(t2[:, 0:clen], o_sb[:, 0:clen], cs[:, 0, :])
                nc.vector.tensor_add(no_[:, 0:clen], t1[:, 0:clen], t2[:, 0:clen])
                return ne, no_

            def proj_rounds(ecols, ocols, xoff, c0, clen, cs_sb, dma_fn):
                """One 512-token chunk of paired (evens, odds) projection +
                rope + repartition DMA."""
                pe = pps.tile([128, 512], F32, tag="pe")
                po = pps.tile([128, 512], F32, tag="po")
                for q4 in range(clen // 256):
                    tcols = (xoff + c0 + q4 * 256, xoff + c0 + q4 * 256 + 256)
                    for cols, dst in ((ecols, pe), (ocols, po)):
                        oap = dst[:, q4 * 256:(q4 + 1) * 256]
                        for p in range(4):
                            dr3(oap, p, wqh_sb, wql_sb, cols,
                                xh_sb, xl_sb, tcols, p == 0, p == 3)
                ne, no_ = rope_chunk(pe, po, cs_sb, c0, clen)
                dma_fn(ne, no_, c0, clen)

            # q: wqk cols [0:512]=all-heads-evens, [512:1024]=all-heads-odds
            for c4 in range(4):
                def q_dma(ne, no_, c0, clen, c4=c4):
                    for j in range(4):
                        nc.gpsimd.dma_start(
                            out=qTg[c4][0:32, j, c0:c0 + clen],
                            in_=ne[j * 32:(j + 1) * 32, 0:clen])
                        nc.gpsimd.dma_start(
                            out=qTg[c4][32:64, j, c0:c0 + clen],
                            in_=no_[j * 32:(j + 1) * 32, 0:clen])
                for ch in range(2):
                    proj_rounds((c4 * 128, (c4 + 1) * 128),
                                (512 + c4 * 128, 512 + (c4 + 1) * 128),
                                WIN, ch * 512, 512, cq_sb, q_dma)

            # k: wqk cols [1024:1152]=kv evens, [1152:1280]=kv odds, full TH
            def k_dma(ne, no_, c0, clen):
                for g in range(KV):
                    nc.gpsimd.dma_start(
                        out=kT[g][0:32, c0:c0 + clen],
                        in_=ne[g * 32:(g + 1) * 32, 0:clen])
                    nc.gpsimd.dma_start(
                        out=kT[g][32:64, c0:c0 + clen],
                        in_=no_[g * 32:(g + 1) * 32, 0:clen])
            for (c0, clen) in ((0, 512), (512, 512), (1024, 256)):
                proj_rounds((1024, 1152), (1152, 1280), 0, c0, clen,
                            ck_sb, k_dma)

        if dbg is not None:
            nc.sync.dma_start(out=dbg["d_q0"], in_=qTg[0][:, 0, :])
            nc.sync.dma_start(out=dbg["d_q6"], in_=qTg[1][:, 2, :])
            nc.sync.dma_start(out=dbg["d_k0"], in_=kT[0][:])
            nc.sync.dma_start(out=dbg["d_k1"], in_=kT[1][:])
            nc.sync.dma_start(out=dbg["d_v"],
                              in_=v65[:].rearrange("p a b c -> p (a b c)"))

        # ======== phase 2: attention + interleaved output projection ========
        with tc.tile_pool(name="stps", bufs=2, space="PSUM") as stps, \
             tc.tile_pool(name="yups", bufs=1, space="PSUM") as yups, \
             tc.tile_pool(name="ops", bufs=2, space="PSUM") as ops, \
             tc.tile_pool(name="atts", bufs=3) as atts:
            for qb in range(8):
                for g in range(KV):
                    yu = yups.tile([128, 4, 96], F32, tag="yu")
                    for jh in range(2):     # head pairs within the group
                        stq = stps.tile([128, 3, 2, 128], F32, tag="stq")
                        for cc in range(3):
                            nc.tensor.matmul(
                                stq[:, cc, :, :],
                                kT[g][:, (qb + cc) * 128:(qb + cc + 1) * 128],
                                qTg[g][:, 2 * jh:2 * jh + 2,
                                       qb * 128:(qb + 1) * 128],
                                start=True, stop=True)
                        # exp over the (3 blocks x 2 heads) tile
                        pt = atts.tile([128, 3, 2, 128], BF16, tag="pt")
                        nc.scalar.activation(
                            pt[:].rearrange("p a b c -> p (a b c)"),
                            stq[:].rearrange("p a b c -> p (a b c)"),
                            mybir.ActivationFunctionType.Exp, scale=0.125)
                        # multiplicative 0/1 band masks on the edge blocks
                        s0 = 0 if qb <= 1 else 1
                        edges = dataclasses.replace(
                            pt[:], ap=[pt.ap[0], [512, 2], [128, 2], [1, 128]])
                        mop = dataclasses.replace(
                            mb_sb[:],
                            ap=[mb_sb.ap[0], [128 * (2 - s0), 2], [0, 2],
                                [1, 128]],
                            offset=mb_sb.offset + 128 * s0)
                        nc.vector.tensor_mul(edges, edges, mop)
                        if qb == 0:
                            m1 = dataclasses.replace(
                                mb_sb[:, 3, :],
                                ap=[mb_sb.ap[0], [0, 2], [1, 128]])
                            nc.vector.tensor_mul(pt[:, 1, :, :],
                                                 pt[:, 1, :, :], m1)
                        if dbg is not None and qb == 3 and g == 1 and jh == 1:
                            nc.sync.dma_start(
                                out=dbg["d_pt"],
                                in_=pt[:].rearrange("p a b c -> p (a b c)"))
                        # att@v flipped: stat = pt block, mov = v (+ones col)
                        for j2 in range(2):
                            for cc in range(3):
                                nc.tensor.matmul(
                                    yu[:, 2 * jh + j2, 0:65],
                                    pt[:, cc, j2, :],
                                    v65[:, qb + cc, g, :],
                                    start=(cc == 0), stop=(cc == 2))
                    # denominators: column 64 -> reciprocal -> normalize
                    rsb = atts.tile([128, 4], F32, tag="rsb")
                    nc.vector.reciprocal(rsb[:], yu[:, :, 64])
                    yv = atts.tile([128, 4, 64], BF16, tag="yv")
                    rbc = dataclasses.replace(
                        rsb[:], ap=[rsb.ap[0], [1, 4], [0, 64]])
                    nc.vector.tensor_mul(yv[:], yu[:, :, 0:64], rbc)
                    if dbg is not None and qb == 3 and g == 1:
                        nc.sync.dma_start(out=dbg["d_rs"], in_=rsb[:])
                        nc.sync.dma_start(
                            out=dbg["d_yv"],
                            in_=yv[:].rearrange("p a b -> p (a b)"))
                    # transpose [q, d] -> [d, q] for the output projection
                    tp = yups.tile([128, 2, 128], BF16, tag="tp")
                    for jj in range(2):
                        nc.tensor.transpose(
                            tp[:, jj, :],
                            yv[:].rearrange("p a b -> p (a b)")[
                                :, jj * 128:(jj + 1) * 128],
                            idt_sb[:])
                    ydst = dataclasses.replace(
                        yT[:, 2 * g, qb * 128:(qb + 1) * 128],
                        ap=[yT.ap[0], [TL, 2], [1, 128]])
                    nc.vector.tensor_copy(ydst, tp[:])
                # output projection for this qb (t-tile == qb)
                o_sb = atts.tile([128, C], BF16, tag="o_sb")
                for oc in range(2):
                    op = ops.tile([128, 512], F32, tag="op")
                    for pr in range(8):
                        nc.tensor.matmul(
                            op[:],
                            yT[:, pr, qb * 128:(qb + 1) * 128],
                            wp_sb[:, pr, oc * 512:(oc + 1) * 512],
                            start=(pr == 0), stop=(pr == 7))
                    nc.vector.tensor_copy(o_sb[:, oc * 512:(oc + 1) * 512],
                                          op[:])
                nc.gpsimd.dma_start(out=out[qb * 128:(qb + 1) * 128, :],
                                    in_=o_sb[:])
            if dbg is not None:
                nc.sync.dma_start(out=dbg["d_yT"],
                                  in_=yT[:].rearrange("p a b -> p (a b)"))


_PROGRAM_CACHE = {}


def _get_program():
    if "nc" not in _PROGRAM_CACHE:
        _PROGRAM_CACHE["nc"] = _build_program()
    return _PROGRAM_CACHE["nc"]


def _hi_lo(a):
    hi = a.astype(F8)
    lo = (a - hi.astype(np.float32)).astype(F8)
    return hi, lo


def _interleave(a):
    """(1024, N) channel-major -> (128, 8, N) partition-major layout."""
    n = a.shape[1]
    return np.ascontiguousarray(a.reshape(8, 128, n).transpose(1, 0, 2))


def prepare_in_maps(x, freqs_cos, freqs_sin, w_attn, b_attn, w_proj, b_proj):
    x = np.asarray(x, dtype=np.float32)
    freqs_cos = np.asarray(freqs_cos, dtype=np.float32)
    freqs_sin = np.asarray(freqs_sin, dtype=np.float32)
    w_attn = np.asarray(w_attn, dtype=np.float32)
    b_attn = np.asarray(b_attn, dtype=np.float32)
    w_proj = np.asarray(w_proj, dtype=np.float32)
    assert not np.any(b_attn), "kernel assumes zero qkv bias"

    # q/k channel permutation: evens block then odds block, head-major
    qch = np.arange(H * HD).reshape(H, 32, 2)
    q_perm = np.concatenate([qch[:, :, 0].reshape(-1), qch[:, :, 1].reshape(-1)])
    kch = H * HD + np.arange(KV * HD).reshape(KV, 32, 2)
    k_perm = np.concatenate([kch[:, :, 0].reshape(-1), kch[:, :, 1].reshape(-1)])
    wqk = np.ascontiguousarray(
        w_attn[np.concatenate([q_perm, k_perm])].T) * WS     # (1024, 1280)
    wqk_h, wqk_l = _hi_lo(wqk)
    wv_f = np.ascontiguousarray(w_attn[(H + KV) * HD:].T) * WS
    wv_h, wv_l = _hi_lo(wv_f)
    wp_h = np.ascontiguousarray(w_proj.T).astype(BF)

    cos4 = np.tile(freqs_cos.T, (4, 1)).astype(np.float32)    # (128, T)
    sin4 = np.tile(freqs_sin.T, (4, 1)).astype(np.float32)

    # multiplicative mask bank (0/1), slots:
    #   0: cc0 for qb<=1   1: cc0 standard   2: cc2   3: cc1 for qb0
    k = np.arange(128)[:, None]
    q = np.arange(128)[None, :]
    upper = (k > q).astype(np.float32)
    lower = (k <= q).astype(np.float32)

    def mask_bank(first_half):
        a = np.zeros((128, 128), np.float32) if first_half else upper
        d = np.zeros((128, 128), np.float32) if first_half else \
            np.ones((128, 128), np.float32)
        return np.stack([a, upper, lower, d], axis=1).astype(BF)

    idt = np.eye(128, dtype=np.float32).astype(BF)

    in_maps = []
    for core in range(8):
        b, h = divmod(core, 2)
        t0 = h * TL
        xs = np.zeros((TH, C), dtype=np.float32)
        lo = max(0, t0 - WIN)
        xs[TH - (t0 + TL - lo):] = x[b, lo:t0 + TL]
        xT = np.ascontiguousarray(xs.T)            # (1024, 1280)
        xT_h, xT_l = _hi_lo(xT)
        cpad = np.zeros((128, TH), dtype=np.float32)
        spad = np.zeros((128, TH), dtype=np.float32)
        cpad[:, TH - (t0 + TL - lo):] = cos4[:, lo:t0 + TL]
        spad[:, TH - (t0 + TL - lo):] = sin4[:, lo:t0 + TL]
        in_maps.append({
            "xh": _interleave(xT_h), "xl": _interleave(xT_l),
            "wqkh": _interleave(wqk_h), "wqkl": _interleave(wqk_l),
            "wvh": _interleave(wv_h), "wvl": _interleave(wv_l),
            "wp": _interleave(wp_h),
            "cq": np.ascontiguousarray(
                np.stack([cos4[:, t0:t0 + TL],
                          sin4[:, t0:t0 + TL]], axis=1)).astype(BF),
            "ck": np.ascontiguousarray(
                np.stack([cpad, spad], axis=1)).astype(BF),
            "mbm": mask_bank(h == 0), "idt": idt,
        })

    return in_maps


def kernel(**inputs):
    in_maps = prepare_in_maps(**inputs)
    nc = _get_program()
    res = run_bass_kernel_spmd(nc, in_maps, list(range(8)))
    return _gather(res, np.asarray(inputs["b_proj"], dtype=np.float32))


def _gather(res, b_proj):
    out = np.empty((B, T, C), dtype=np.float32)
    for core in range(8):
        b, h = divmod(core, 2)
        out[b, h * TL:(h + 1) * TL] = np.asarray(
            res.results[core]["out"], dtype=np.float32)
    if np.any(b_proj):
        out += b_proj
    return out


# revision 16
# speedup vs baseline: 1.4534x; 1.4534x over previous
"""Sliding-window GQA causal self-attention block for 8 trn2 NeuronCores.

Sharding: batch (4) x T-halves (2) -> 8 cores, no collectives. Each core gets
x.T for its T-half plus a 256-row key/value halo and computes its (1024, 1024)
slice of the output.

v3 design notes (cost-model driven):
- qkv projection runs as fp8e4 hi+lo DoubleRow matmuls (x and w split into
  fp8 hi/lo pairs on the host; 3 DR terms replace 4 bf16 matmuls per
  256-channel pair -> 25% fewer PE rows at bf16-level accuracy).
- phase 1 runs in 512-token chunks with double-buffered PSUM so rope
  (ACT/DVE) overlaps the next chunk's matmuls.
- scores are j2-batched: group-major q layout lets one bf16 matmul produce
  [128 keys, 2 heads x 128 queries] (N=256), halving score instruction count.
- band masks are multiplicative 0/1 bf16 DVE multiplies on the exp'd tile
  (stride-0 broadcast over the head pair).
- att@v is "flipped": stationary = exp'd scores, moving = v (65 cols incl a
  ones column), so cost is 65 rows per key block and the softmax denominator
  lands as a per-query PSUM column -> [128, 4] reciprocal + stride-0
  broadcast normalize on DVE.
- normalized y ([q, d] layout) is transposed back to [d, q] via PE transpose
  for the row-major output projection (bf16).
- one DMA per input tensor (host pre-transposes layouts); qT/kT/out DMAs go
  through the idle Pool (gpsimd) queue.
"""

import dataclasses

import numpy as np
import ml_dtypes

import concourse.bass as bass
import concourse.mybir as mybir
import concourse.tile as tile
from concourse import bacc
from concourse.bass_utils import run_bass_kernel_spmd

F8 = ml_dtypes.float8_e4m3fn
BF = ml_dtypes.bfloat16
F32 = mybir.dt.float32
BF16 = mybir.dt.bfloat16
FP8 = mybir.dt.float8e4
DR = mybir.MatmulPerfMode.DoubleRow

B, T, C = 4, 2048, 1024
H, KV, HD = 16, 4, 64
WIN = 256
TL = T // 2            # 1024 own rows per core
TH = TL + WIN          # 1280 with halo
WS = 32.0              # host-side weight prescale for fp8 dynamic range


def _build_program():
    nc = bacc.Bacc("TRN2", target_bir_lowering=False, debug=False, num_devices=8)
    dt = mybir.dt
    xh = nc.dram_tensor("xh", [128, 8, TH], dt.float8e4, kind="ExternalInput").ap()
    xl = nc.dram_tensor("xl", [128, 8, TH], dt.float8e4, kind="ExternalInput").ap()
    wqkh = nc.dram_tensor("wqkh", [128, 8, 1280], dt.float8e4, kind="ExternalInput").ap()
    wqkl = nc.dram_tensor("wqkl", [128, 8, 1280], dt.float8e4, kind="ExternalInput").ap()
    wvh = nc.dram_tensor("wvh", [128, 8, 256], dt.float8e4, kind="ExternalInput").ap()
    wvl = nc.dram_tensor("wvl", [128, 8, 256], dt.float8e4, kind="ExternalInput").ap()
    wp = nc.dram_tensor("wp", [128, 8, C], dt.bfloat16, kind="ExternalInput").ap()
    cq = nc.dram_tensor("cq", [128, 2, TL], dt.bfloat16, kind="ExternalInput").ap()
    ck = nc.dram_tensor("ck", [128, 2, TH], dt.bfloat16, kind="ExternalInput").ap()
    mbm = nc.dram_tensor("mbm", [128, 4, 128], dt.bfloat16, kind="ExternalInput").ap()
    idt = nc.dram_tensor("idt", [128, 128], dt.bfloat16, kind="ExternalInput").ap()
    out = nc.dram_tensor("out", [TL, C], dt.bfloat16, kind="ExternalOutput").ap()

    with tile.TileContext(nc) as tc:
        _kernel_body(tc, nc, xh, xl, wqkh, wqkl, wvh, wvl, wp, cq, ck,
                     mbm, idt, out)
    nc.compile()
    return nc


def _kernel_body(tc, nc, xh, xl, wqkh, wqkl, wvh, wvl, wp, cq, ck,
                 mbm, idt, out, dbg=None):
    import contextlib
    ctx = contextlib.ExitStack()
    with ctx:
        consts = ctx.enter_context(tc.tile_pool(name="consts", bufs=1))
        persist = ctx.enter_context(tc.tile_pool(name="persist", bufs=1))

        # ---- load persistent inputs: one DMA per tensor, spread queues ----
        xh_sb = persist.tile([128, 8, TH], FP8, tag="xh")
        xl_sb = persist.tile([128, 8, TH], FP8, tag="xl")
        wvh_sb = persist.tile([128, 8, 256], FP8, tag="wvh")
        wvl_sb = persist.tile([128, 8, 256], FP8, tag="wvl")
        wqh_sb = persist.tile([128, 8, 1280], FP8, tag="wqh")
        wql_sb = persist.tile([128, 8, 1280], FP8, tag="wql")
        wp_sb = persist.tile([128, 8, C], BF16, tag="wp")
        cq_sb = consts.tile([128, 2, TL], BF16)
        ck_sb = consts.tile([128, 2, TH], BF16)
        mb_sb = consts.tile([128, 4, 128], BF16)
        idt_sb = consts.tile([128, 128], BF16)
        nc.sync.dma_start(out=xh_sb[:], in_=xh)
        nc.sync.dma_start(out=xl_sb[:], in_=xl)
        nc.scalar.dma_start(out=wvh_sb[:], in_=wvh)
        nc.scalar.dma_start(out=wvl_sb[:], in_=wvl)
        nc.sync.dma_start(out=wqh_sb[:], in_=wqkh)
        nc.sync.dma_start(out=wql_sb[:], in_=wqkl)
        nc.scalar.dma_start(out=wp_sb[:], in_=wp)
        nc.scalar.dma_start(out=cq_sb[:], in_=cq)
        nc.scalar.dma_start(out=ck_sb[:], in_=ck)
        nc.scalar.dma_start(out=mb_sb[:], in_=mbm)
        nc.scalar.dma_start(out=idt_sb[:], in_=idt)

        # persistent compute tensors
        qTall = persist.tile([64, KV, 4, TL], BF16, tag="qTall")
        kTall = persist.tile([64, KV, TH], BF16, tag="kTall")
        v65 = persist.tile([128, 10, KV, 65], BF16, tag="v65")
        yT = persist.tile([128, 8, TL], BF16, tag="yT")

        def dr3(out_ap, p, stat_h, stat_l, stat_cols, mov_h, mov_l, mov_cols,
                first, last):
            """Three hi/lo DoubleRow terms for chunk pair p (contraction
            channels [256p, 256p+256))."""
            sh = stat_h[:, 2 * p:2 * p + 2, stat_cols[0]:stat_cols[1]]
            sl = stat_l[:, 2 * p:2 * p + 2, stat_cols[0]:stat_cols[1]]
            mh = mov_h[:, 2 * p:2 * p + 2, mov_cols[0]:mov_cols[1]]
            ml = mov_l[:, 2 * p:2 * p + 2, mov_cols[0]:mov_cols[1]]
            nc.tensor.matmul(out_ap, sh, mh, start=first, stop=False,
                             perf_mode=DR)
            nc.tensor.matmul(out_ap, sh, ml, start=False, stop=False,
                             perf_mode=DR)
            nc.tensor.matmul(out_ap, sl, mh, start=False, stop=last,
                             perf_mode=DR)

        # ======== phase 1: qkv projection + rope (512-token chunks) ========
        with tc.tile_pool(name="pps", bufs=2, space="PSUM") as pps, \
             tc.tile_pool(name="vps", bufs=2, space="PSUM") as vps, \
             tc.tile_pool(name="ropes", bufs=2) as ropes:

            # v first: needs only x + wv
            for tcn in range(10):
                pv = vps.tile([128, 256], F32, tag="pv")
                tc_cols = (tcn * 128, (tcn + 1) * 128)
                for p in range(4):
                    dr3(pv[:], p, xh_sb, xl_sb, tc_cols,
                        wvh_sb, wvl_sb, (0, 256), p == 0, p == 3)
                v4 = v65[:, tcn, :, 0:64]
                nc.scalar.mul(v4, pv[:].rearrange("p (g c) -> p g c", c=64),
                              1.0 / WS)
            nc.vector.memset(v65[:, :, :, 64:65], 1.0)

            def rope_chunk(pe, po, cs_sb, c0, clen):
                e_sb = ropes.tile([128, 512], BF16, tag="e_sb")
                o_sb = ropes.tile([128, 512], BF16, tag="o_sb")
                nc.scalar.mul(e_sb[:, 0:clen], pe[:, 0:clen], 1.0 / WS)
                nc.scalar.mul(o_sb[:, 0:clen], po[:, 0:clen], 1.0 / WS)
                ne = ropes.tile([128, 512], BF16, tag="r0")
                no_ = ropes.tile([128, 512], BF16, tag="r1")
                t1 = ropes.tile([128, 512], BF16, tag="r2")
                t2 = ropes.tile([128, 512], BF16, tag="r3")
                cs = cs_sb[:, :, c0:c0 + clen]
                nc.vector.tensor_mul(t1[:, 0:clen], e_sb[:, 0:clen], cs[:, 0, :])
                nc.vector.tensor_mul(t2[:, 0:clen], o_sb[:, 0:clen], cs[:, 1, :])
                nc.vector.tensor_sub(ne[:, 0:clen], t1[:, 0:clen], t2[:, 0:clen])
                nc.vector.tensor_mul(t1[:, 0:clen], e_sb[:, 0:clen], cs[:, 1, :])
                nc.vector.tensor_mul(t2[:, 0:clen], o_sb[:, 0:clen], cs[:, 0, :])
                nc.vector.tensor_add(no_[:, 0:clen], t1[:, 0:clen], t2[:, 0:clen])
                return ne, no_

            def proj_rounds(ecols, ocols, xoff, c0, clen, cs_sb, dma_fn):
                """One 512-token chunk of paired (evens, odds) projection +
                rope + repartition DMA."""
                pe = pps.tile([128, 512], F32, tag="pe")
                po = pps.tile([128, 512], F32, tag="po")
                for q4 in range(clen // 256):
                    tcols = (xoff + c0 + q4 * 256, xoff + c0 + q4 * 256 + 256)
                    for cols, dst in ((ecols, pe), (ocols, po)):
                        oap = dst[:, q4 * 256:(q4 + 1) * 256]
                        for p in range(4):
                            dr3(oap, p, wqh_sb, wql_sb, cols,
                                xh_sb, xl_sb, tcols, p == 0, p == 3)
                ne, no_ = rope_chunk(pe, po, cs_sb, c0, clen)
                dma_fn(ne, no_, c0, clen)

            # q: wqk cols [0:512]=all-heads-evens, [512:1024]=all-heads-odds
            for c4 in range(4):
                def q_dma(ne, no_, c0, clen, c4=c4):
                    nc.sync.dma_start(
                        out=qTall[0:32, c4, :, c0:c0 + clen],
                        in_=ne[:, 0:clen])
                    nc.scalar.dma_start(
                        out=qTall[32:64, c4, :, c0:c0 + clen],
                        in_=no_[:, 0:clen])
                for ch in range(2):
                    proj_rounds((c4 * 128, (c4 + 1) * 128),
                                (512 + c4 * 128, 512 + (c4 + 1) * 128),
                                WIN, ch * 512, 512, cq_sb, q_dma)

            # k: wqk cols [1024:1152]=kv evens, [1152:1280]=kv odds, full TH
            def k_dma(ne, no_, c0, clen):
                nc.sync.dma_start(out=kTall[0:32, :, c0:c0 + clen],
                                  in_=ne[:, 0:clen])
                nc.scalar.dma_start(out=kTall[32:64, :, c0:c0 + clen],
                                    in_=no_[:, 0:clen])
            for (c0, clen) in ((0, 512), (512, 512), (1024, 256)):
                proj_rounds((1024, 1152), (1152, 1280), 0, c0, clen,
                            ck_sb, k_dma)

        if dbg is not None:
            nc.sync.dma_start(out=dbg["d_q0"], in_=qTall[:, 0, 0, :])
            nc.sync.dma_start(out=dbg["d_q6"], in_=qTall[:, 1, 2, :])
            nc.sync.dma_start(out=dbg["d_k0"], in_=kTall[:, 0, :])
            nc.sync.dma_start(out=dbg["d_k1"], in_=kTall[:, 1, :])
            nc.sync.dma_start(out=dbg["d_v"],
                              in_=v65[:].rearrange("p a b c -> p (a b c)"))

        # ======== phase 2: attention + interleaved output projection ========
        with tc.tile_pool(name="stps", bufs=2, space="PSUM") as stps, \
             tc.tile_pool(name="yups", bufs=1, space="PSUM") as yups, \
             tc.tile_pool(name="ops", bufs=2, space="PSUM") as ops, \
             tc.tile_pool(name="atts", bufs=3) as atts:
            for qb in range(8):
                for g in range(KV):
                    yu = yups.tile([128, 4, 96], F32, tag="yu")
                    for jh in range(2):     # head pairs within the group
                        stq = stps.tile([128, 3, 2, 128], F32, tag="stq")
                        for cc in range(3):
                            nc.tensor.matmul(
                                stq[:, cc, :, :],
                                kTall[:, g,
                                      (qb + cc) * 128:(qb + cc + 1) * 128],
                                qTall[:, g, 2 * jh:2 * jh + 2,
                                      qb * 128:(qb + 1) * 128],
                                start=True, stop=True)
                        # exp over the (3 blocks x 2 heads) tile
                        pt = atts.tile([128, 3, 2, 128], BF16, tag="pt")
                        nc.scalar.activation(
                            pt[:].rearrange("p a b c -> p (a b c)"),
                            stq[:].rearrange("p a b c -> p (a b c)"),
                            mybir.ActivationFunctionType.Exp, scale=0.125)
                        # multiplicative 0/1 band masks on the edge blocks
                        s0 = 0 if qb <= 1 else 1
                        edges = dataclasses.replace(
                            pt[:], ap=[pt.ap[0], [512, 2], [128, 2], [1, 128]])
                        mop = dataclasses.replace(
                            mb_sb[:],
                            ap=[mb_sb.ap[0], [128 * (2 - s0), 2], [0, 2],
                                [1, 128]],
                            offset=mb_sb.offset + 128 * s0)
                        nc.vector.tensor_mul(edges, edges, mop)
                        if qb == 0:
                            m1 = dataclasses.replace(
                                mb_sb[:, 3, :],
                                ap=[mb_sb.ap[0], [0, 2], [1, 128]])
                            nc.vector.tensor_mul(pt[:, 1, :, :],
                                                 pt[:, 1, :, :], m1)
                        if dbg is not None and qb == 3 and g == 1 and jh == 1:
                            nc.sync.dma_start(
                                out=dbg["d_pt"],
                                in_=pt[:].rearrange("p a b c -> p (a b c)"))
                        # att@v flipped: stat = pt block, mov = v (+ones col)
                        for j2 in range(2):
                            for cc in range(3):
                                nc.tensor.matmul(
                                    yu[:, 2 * jh + j2, 0:65],
                                    pt[:, cc, j2, :],
                                    v65[:, qb + cc, g, :],
                                    start=(cc == 0), stop=(cc == 2))
                    # denominators: column 64 -> reciprocal -> normalize
                    rsb = atts.tile([128, 4], F32, tag="rsb")
                    nc.vector.reciprocal(rsb[:], yu[:, :, 64])
                    yv = atts.tile([128, 4, 64], BF16, tag="yv")
                    rbc = dataclasses.replace(
                        rsb[:], ap=[rsb.ap[0], [1, 4], [0, 64]])
                    nc.vector.tensor_mul(yv[:], yu[:, :, 0:64], rbc)
                    if dbg is not None and qb == 3 and g == 1:
                        nc.sync.dma_start(out=dbg["d_rs"], in_=rsb[:])
                        nc.sync.dma_start(
                            out=dbg["d_yv"],
                            in_=yv[:].rearrange("p a b -> p (a b)"))
                    # transpose [q, d] -> [d, q] for the output projection
                    tp = yups.tile([128, 2, 128], BF16, tag="tp")
                    for jj in range(2):
                        nc.tensor.transpose(
                            tp[:, jj, :],
                            yv[:].rearrange("p a b -> p (a b)")[
                                :, jj * 128:(jj + 1) * 128],
                            idt_sb[:])
                    ydst = dataclasses.replace(
                        yT[:, 2 * g, qb * 128:(qb + 1) * 128],
                        ap=[yT.ap[0], [TL, 2], [1, 128]])
                    nc.vector.tensor_copy(ydst, tp[:])
                # output projection for this qb (t-tile == qb)
                o_sb = atts.tile([128, C], BF16, tag="o_sb")
                for oc in range(2):
                    op = ops.tile([128, 512], F32, tag="op")
                    for pr in range(8):
                        nc.tensor.matmul(
                            op[:],
                            yT[:, pr, qb * 128:(qb + 1) * 128],
                            wp_sb[:, pr, oc * 512:(oc + 1) * 512],
                            start=(pr == 0), stop=(pr == 7))
                    nc.vector.tensor_copy(o_sb[:, oc * 512:(oc + 1) * 512],
                                          op[:])
                nc.scalar.dma_start(out=out[qb * 128:(qb + 1) * 128, :],
                                    in_=o_sb[:])
            if dbg is not None:
                nc.sync.dma_start(out=dbg["d_yT"],
                                  in_=yT[:].rearrange("p a b -> p (a b)"))


_PROGRAM_CACHE = {}


def _get_program():
    if "nc" not in _PROGRAM_CACHE:
        _PROGRAM_CACHE["nc"] = _build_program()
    return _PROGRAM_CACHE["nc"]


def _hi_lo(a):
    hi = a.astype(F8)
    lo = (a - hi.astype(np.float32)).astype(F8)
    return hi, lo


def _interleave(a):
    """(1024, N) channel-major -> (128, 8, N) partition-major layout."""
    n = a.shape[1]
    return np.ascontiguousarray(a.reshape(8, 128, n).transpose(1, 0, 2))


def prepare_in_maps(x, freqs_cos, freqs_sin, w_attn, b_attn, w_proj, b_proj):
    x = np.asarray(x, dtype=np.float32)
    freqs_cos = np.asarray(freqs_cos, dtype=np.float32)
    freqs_sin = np.asarray(freqs_sin, dtype=np.float32)
    w_attn = np.asarray(w_attn, dtype=np.float32)
    b_attn = np.asarray(b_attn, dtype=np.float32)
    w_proj = np.asarray(w_proj, dtype=np.float32)
    assert not np.any(b_attn), "kernel assumes zero qkv bias"

    # q/k channel permutation: evens block then odds block; within each
    # 128-wide group block the order is (d-pair, head) so the rope output
    # repartitions to [32, 4, T] with a single DMA.
    q_ev, q_od = [], []
    for c4 in range(4):
        q_ev += [(4 * c4 + j) * HD + 2 * p for p in range(32) for j in range(4)]
        q_od += [(4 * c4 + j) * HD + 2 * p + 1 for p in range(32) for j in range(4)]
    k_ev = [H * HD + g * HD + 2 * p for p in range(32) for g in range(KV)]
    k_od = [H * HD + g * HD + 2 * p + 1 for p in range(32) for g in range(KV)]
    q_perm = np.array(q_ev + q_od)
    k_perm = np.array(k_ev + k_od)
    wqk = np.ascontiguousarray(
        w_attn[np.concatenate([q_perm, k_perm])].T) * WS     # (1024, 1280)
    wqk_h, wqk_l = _hi_lo(wqk)
    wv_f = np.ascontiguousarray(w_attn[(H + KV) * HD:].T) * WS
    wv_h, wv_l = _hi_lo(wv_f)
    wp_h = np.ascontiguousarray(w_proj.T).astype(BF)

    # row r of a projection tile is d-pair r//4 (d-pair-major order)
    cos4 = np.repeat(freqs_cos.T, 4, axis=0).astype(np.float32)   # (128, T)
    sin4 = np.repeat(freqs_sin.T, 4, axis=0).astype(np.float32)

    # multiplicative mask bank (0/1), slots:
    #   0: cc0 for qb<=1   1: cc0 standard   2: cc2   3: cc1 for qb0
    k = np.arange(128)[:, None]
    q = np.arange(128)[None, :]
    upper = (k > q).astype(np.float32)
    lower = (k <= q).astype(np.float32)

    def mask_bank(first_half):
        a = np.zeros((128, 128), np.float32) if first_half else upper
        d = np.zeros((128, 128), np.float32) if first_half else \
            np.ones((128, 128), np.float32)
        return np.stack([a, upper, lower, d], axis=1).astype(BF)

    idt = np.eye(128, dtype=np.float32).astype(BF)

    in_maps = []
    for core in range(8):
        b, h = divmod(core, 2)
        t0 = h * TL
        xs = np.zeros((TH, C), dtype=np.float32)
        lo = max(0, t0 - WIN)
        xs[TH - (t0 + TL - lo):] = x[b, lo:t0 + TL]
        xT = np.ascontiguousarray(xs.T)            # (1024, 1280)
        xT_h, xT_l = _hi_lo(xT)
        cpad = np.zeros((128, TH), dtype=np.float32)
        spad = np.zeros((128, TH), dtype=np.float32)
        cpad[:, TH - (t0 + TL - lo):] = cos4[:, lo:t0 + TL]
        spad[:, TH - (t0 + TL - lo):] = sin4[:, lo:t0 + TL]
        in_maps.append({
            "xh": _interleave(xT_h), "xl": _interleave(xT_l),
            "wqkh": _interleave(wqk_h), "wqkl": _interleave(wqk_l),
            "wvh": _interleave(wv_h), "wvl": _interleave(wv_l),
            "wp": _interleave(wp_h),
            "cq": np.ascontiguousarray(
                np.stack([cos4[:, t0:t0 + TL],
                          sin4[:, t0:t0 + TL]], axis=1)).astype(BF),
            "ck": np.ascontiguousarray(
                np.stack([cpad, spad], axis=1)).astype(BF),
            "mbm": mask_bank(h == 0), "idt": idt,
        })

    return in_maps


def kernel(**inputs):
    in_maps = prepare_in_maps(**inputs)
    nc = _get_program()
    res = run_bass_kernel_spmd(nc, in_maps, list(range(8)))
    return _gather(res, np.asarray(inputs["b_proj"], dtype=np.float32))


def _gather(res, b_proj):
    out = np.empty((B, T, C), dtype=np.float32)
    for core in range(8):
        b, h = divmod(core, 2)
        out[b, h * TL:(h + 1) * TL] = np.asarray(
            res.results[core]["out"], dtype=np.float32)
    if np.any(b_proj):
        out += b_proj
    return out


# revision 17
# speedup vs baseline: 1.4644x; 1.0076x over previous
"""Sliding-window GQA causal self-attention block for 8 trn2 NeuronCores.

Sharding: batch (4) x T-halves (2) -> 8 cores, no collectives. Each core gets
x.T for its T-half plus a 256-row key/value halo and computes its (1024, 1024)
slice of the output.

v3 design notes (cost-model driven):
- qkv projection runs as fp8e4 hi+lo DoubleRow matmuls (x and w split into
  fp8 hi/lo pairs on the host; 3 DR terms replace 4 bf16 matmuls per
  256-channel pair -> 25% fewer PE rows at bf16-level accuracy).
- phase 1 runs in 512-token chunks with double-buffered PSUM so rope
  (ACT/DVE) overlaps the next chunk's matmuls.
- scores are j2-batched: group-major q layout lets one bf16 matmul produce
  [128 keys, 2 heads x 128 queries] (N=256), halving score instruction count.
- band masks are multiplicative 0/1 bf16 DVE multiplies on the exp'd tile
  (stride-0 broadcast over the head pair).
- att@v is "flipped": stationary = exp'd scores, moving = v (65 cols incl a
  ones column), so cost is 65 rows per key block and the softmax denominator
  lands as a per-query PSUM column -> [128, 4] reciprocal + stride-0
  broadcast normalize on DVE.
- normalized y ([q, d] layout) is transposed back to [d, q] via PE transpose
  for the row-major output projection (bf16).
- one DMA per input tensor (host pre-transposes layouts); qT/kT/out DMAs go
  through the idle Pool (gpsimd) queue.
"""

import dataclasses

import numpy as np
import ml_dtypes

import concourse.bass as bass
import concourse.mybir as mybir
import concourse.tile as tile
from concourse import bacc
from concourse.bass_utils import run_bass_kernel_spmd

F8 = ml_dtypes.float8_e4m3fn
BF = ml_dtypes.bfloat16
F32 = mybir.dt.float32
BF16 = mybir.dt.bfloat16
FP8 = mybir.dt.float8e4
DR = mybir.MatmulPerfMode.DoubleRow

B, T, C = 4, 2048, 1024
H, KV, HD = 16, 4, 64
WIN = 256
TL = T // 2            # 1024 own rows per core
TH = TL + WIN          # 1280 with halo
WS = 32.0              # host-side weight prescale for fp8 dynamic range


def _build_program():
    nc = bacc.Bacc("TRN2", target_bir_lowering=False, debug=False, num_devices=8)
    dt = mybir.dt
    xh = nc.dram_tensor("xh", [128, 8, TH], dt.float8e4, kind="ExternalInput").ap()
    xl = nc.dram_tensor("xl", [128, 8, TH], dt.float8e4, kind="ExternalInput").ap()
    wqkh = nc.dram_tensor("wqkh", [128, 8, 1280], dt.float8e4, kind="ExternalInput").ap()
    wqkl = nc.dram_tensor("wqkl", [128, 8, 1280], dt.float8e4, kind="ExternalInput").ap()
    wvh = nc.dram_tensor("wvh", [128, 8, 256], dt.float8e4, kind="ExternalInput").ap()
    wvl = nc.dram_tensor("wvl", [128, 8, 256], dt.float8e4, kind="ExternalInput").ap()
    wp = nc.dram_tensor("wp", [128, 8, C], dt.bfloat16, kind="ExternalInput").ap()
    cq = nc.dram_tensor("cq", [128, 2, TL], dt.bfloat16, kind="ExternalInput").ap()
    ck = nc.dram_tensor("ck", [128, 2, TH], dt.bfloat16, kind="ExternalInput").ap()
    mbm = nc.dram_tensor("mbm", [128, 4, 128], dt.bfloat16, kind="ExternalInput").ap()
    idt = nc.dram_tensor("idt", [128, 128], dt.bfloat16, kind="ExternalInput").ap()
    out = nc.dram_tensor("out", [TL, C], dt.bfloat16, kind="ExternalOutput").ap()

    with tile.TileContext(nc) as tc:
        _kernel_body(tc, nc, xh, xl, wqkh, wqkl, wvh, wvl, wp, cq, ck,
                     mbm, idt, out)
    nc.compile()
    return nc


def _kernel_body(tc, nc, xh, xl, wqkh, wqkl, wvh, wvl, wp, cq, ck,
                 mbm, idt, out, dbg=None):
    import contextlib
    ctx = contextlib.ExitStack()
    with ctx:
        consts = ctx.enter_context(tc.tile_pool(name="consts", bufs=1))
        persist = ctx.enter_context(tc.tile_pool(name="persist", bufs=1))

        # ---- load persistent inputs: one DMA per tensor, spread queues ----
        xh_sb = persist.tile([128, 8, TH], FP8, tag="xh")
        xl_sb = persist.tile([128, 8, TH], FP8, tag="xl")
        wvh_sb = persist.tile([128, 8, 256], FP8, tag="wvh")
        wvl_sb = persist.tile([128, 8, 256], FP8, tag="wvl")
        wqh_sb = persist.tile([128, 8, 1280], FP8, tag="wqh")
        wql_sb = persist.tile([128, 8, 1280], FP8, tag="wql")
        wp_sb = persist.tile([128, 8, C], BF16, tag="wp")
        cq_sb = consts.tile([128, 2, TL], BF16)
        ck_sb = consts.tile([128, 2, TH], BF16)
        mb_sb = consts.tile([128, 4, 128], BF16)
        idt_sb = consts.tile([128, 128], BF16)
        nc.scalar.dma_start(out=wvh_sb[:], in_=wvh)
        nc.scalar.dma_start(out=wvl_sb[:], in_=wvl)
        for hf in range(4):   # quarter the big loads so compute starts early
            s = slice(2 * hf, 2 * hf + 2)
            nc.sync.dma_start(out=xh_sb[:, s, :], in_=xh[:, s, :])
            nc.sync.dma_start(out=xl_sb[:, s, :], in_=xl[:, s, :])
        for hf in range(2):
            s = slice(4 * hf, 4 * hf + 4)
            nc.sync.dma_start(out=wqh_sb[:, s, :], in_=wqkh[:, s, :])
            nc.sync.dma_start(out=wql_sb[:, s, :], in_=wqkl[:, s, :])
        nc.scalar.dma_start(out=cq_sb[:], in_=cq)
        nc.scalar.dma_start(out=ck_sb[:], in_=ck)
        nc.scalar.dma_start(out=mb_sb[:], in_=mbm)
        nc.scalar.dma_start(out=idt_sb[:], in_=idt)
        nc.scalar.dma_start(out=wp_sb[:], in_=wp)

        # persistent compute tensors
        qTall = persist.tile([64, KV, 4, TL], BF16, tag="qTall")
        kTall = persist.tile([64, KV, TH], BF16, tag="kTall")
        v65 = persist.tile([128, 10, KV, 65], BF16, tag="v65")
        yT = persist.tile([128, 8, TL], BF16, tag="yT")

        def dr3(out_ap, p, stat_h, stat_l, stat_cols, mov_h, mov_l, mov_cols,
                first, last):
            """Three hi/lo DoubleRow terms for chunk pair p (contraction
            channels [256p, 256p+256))."""
            sh = stat_h[:, 2 * p:2 * p + 2, stat_cols[0]:stat_cols[1]]
            sl = stat_l[:, 2 * p:2 * p + 2, stat_cols[0]:stat_cols[1]]
            mh = mov_h[:, 2 * p:2 * p + 2, mov_cols[0]:mov_cols[1]]
            ml = mov_l[:, 2 * p:2 * p + 2, mov_cols[0]:mov_cols[1]]
            nc.tensor.matmul(out_ap, sh, mh, start=first, stop=False,
                             perf_mode=DR)
            nc.tensor.matmul(out_ap, sh, ml, start=False, stop=False,
                             perf_mode=DR)
            nc.tensor.matmul(out_ap, sl, mh, start=False, stop=last,
                             perf_mode=DR)

        # ======== phase 1: qkv projection + rope (512-token chunks) ========
        with tc.tile_pool(name="pps", bufs=3, space="PSUM") as pps, \
             tc.tile_pool(name="vps", bufs=2, space="PSUM") as vps, \
             tc.tile_pool(name="ropes", bufs=3) as ropes:

            # v first: needs only x + wv
            for tcn in range(10):
                pv = vps.tile([128, 256], F32, tag="pv")
                tc_cols = (tcn * 128, (tcn + 1) * 128)
                for p in range(4):
                    dr3(pv[:], p, xh_sb, xl_sb, tc_cols,
                        wvh_sb, wvl_sb, (0, 256), p == 0, p == 3)
                v4 = v65[:, tcn, :, 0:64]
                nc.scalar.mul(v4, pv[:].rearrange("p (g c) -> p g c", c=64),
                              1.0 / WS)
            nc.vector.memset(v65[:, :, :, 64:65], 1.0)

            def rope_chunk(pe, po, cs_sb, c0, clen):
                e_sb = ropes.tile([128, 512], BF16, tag="e_sb")
                o_sb = ropes.tile([128, 512], BF16, tag="o_sb")
                nc.scalar.mul(e_sb[:, 0:clen], pe[:, 0:clen], 1.0 / WS)
                nc.scalar.mul(o_sb[:, 0:clen], po[:, 0:clen], 1.0 / WS)
                ne = ropes.tile([128, 512], BF16, tag="r0")
                no_ = ropes.tile([128, 512], BF16, tag="r1")
                t1 = ropes.tile([128, 512], BF16, tag="r2")
                t2 = ropes.tile([128, 512], BF16, tag="r3")
                cs = cs_sb[:, :, c0:c0 + clen]
                nc.vector.tensor_mul(t1[:, 0:clen], e_sb[:, 0:clen], cs[:, 0, :])
                nc.vector.tensor_mul(t2[:, 0:clen], o_sb[:, 0:clen], cs[:, 1, :])
                nc.vector.tensor_sub(ne[:, 0:clen], t1[:, 0:clen], t2[:, 0:clen])
                nc.vector.tensor_mul(t1[:, 0:clen], e_sb[:, 0:clen], cs[:, 1, :])
                nc.vector.tensor_mul(t2[:, 0:clen], o_sb[:, 0:clen], cs[:, 0, :])
                nc.vector.tensor_add(no_[:, 0:clen], t1[:, 0:clen], t2[:, 0:clen])
                return ne, no_

            def proj_rounds(ecols, ocols, xoff, c0, clen, cs_sb, dma_fn):
                """One 512-token chunk of paired (evens, odds) projection +
                rope + repartition DMA."""
                pe = pps.tile([128, 512], F32, tag="pe")
                po = pps.tile([128, 512], F32, tag="po")
                for q4 in range(clen // 256):
                    tcols = (xoff + c0 + q4 * 256, xoff + c0 + q4 * 256 + 256)
                    for cols, dst in ((ecols, pe), (ocols, po)):
                        oap = dst[:, q4 * 256:(q4 + 1) * 256]
                        for p in range(4):
                            dr3(oap, p, wqh_sb, wql_sb, cols,
                                xh_sb, xl_sb, tcols, p == 0, p == 3)
                ne, no_ = rope_chunk(pe, po, cs_sb, c0, clen)
                dma_fn(ne, no_, c0, clen)

            # q: wqk cols [0:512]=all-heads-evens, [512:1024]=all-heads-odds
            for c4 in range(4):
                def q_dma(ne, no_, c0, clen, c4=c4):
                    nc.sync.dma_start(
                        out=qTall[0:32, c4, :, c0:c0 + clen],
                        in_=ne[:, 0:clen])
                    nc.scalar.dma_start(
                        out=qTall[32:64, c4, :, c0:c0 + clen],
                        in_=no_[:, 0:clen])
                for ch in range(2):
                    proj_rounds((c4 * 128, (c4 + 1) * 128),
                                (512 + c4 * 128, 512 + (c4 + 1) * 128),
                                WIN, ch * 512, 512, cq_sb, q_dma)

            # k: wqk cols [1024:1152]=kv evens, [1152:1280]=kv odds, full TH
            def k_dma(ne, no_, c0, clen):
                nc.sync.dma_start(out=kTall[0:32, :, c0:c0 + clen],
                                  in_=ne[:, 0:clen])
                nc.scalar.dma_start(out=kTall[32:64, :, c0:c0 + clen],
                                    in_=no_[:, 0:clen])
            for (c0, clen) in ((0, 512), (512, 512), (1024, 256)):
                proj_rounds((1024, 1152), (1152, 1280), 0, c0, clen,
                            ck_sb, k_dma)

        if dbg is not None:
            nc.sync.dma_start(out=dbg["d_q0"], in_=qTall[:, 0, 0, :])
            nc.sync.dma_start(out=dbg["d_q6"], in_=qTall[:, 1, 2, :])
            nc.sync.dma_start(out=dbg["d_k0"], in_=kTall[:, 0, :])
            nc.sync.dma_start(out=dbg["d_k1"], in_=kTall[:, 1, :])
            nc.sync.dma_start(out=dbg["d_v"],
                              in_=v65[:].rearrange("p a b c -> p (a b c)"))

        # ======== phase 2: attention + interleaved output projection ========
        with tc.tile_pool(name="stps", bufs=2, space="PSUM") as stps, \
             tc.tile_pool(name="yups", bufs=1, space="PSUM") as yups, \
             tc.tile_pool(name="ops", bufs=2, space="PSUM") as ops, \
             tc.tile_pool(name="atts", bufs=3) as atts:
            for qb in range(8):
                for g in range(KV):
                    yu = yups.tile([128, 4, 96], F32, tag="yu")
                    for jh in range(2):     # head pairs within the group
                        stq = stps.tile([128, 3, 2, 128], F32, tag="stq")
                        for cc in range(3):
                            nc.tensor.matmul(
                                stq[:, cc, :, :],
                                kTall[:, g,
                                      (qb + cc) * 128:(qb + cc + 1) * 128],
                                qTall[:, g, 2 * jh:2 * jh + 2,
                                      qb * 128:(qb + 1) * 128],
                                start=True, stop=True)
                        # exp over the (3 blocks x 2 heads) tile
                        pt = atts.tile([128, 3, 2, 128], BF16, tag="pt")
                        nc.scalar.activation(
                            pt[:].rearrange("p a b c -> p (a b c)"),
                            stq[:].rearrange("p a b c -> p (a b c)"),
                            mybir.ActivationFunctionType.Exp, scale=0.125)
                        # multiplicative 0/1 band masks on the edge blocks
                        s0 = 0 if qb <= 1 else 1
                        edges = dataclasses.replace(
                            pt[:], ap=[pt.ap[0], [512, 2], [128, 2], [1, 128]])
                        mop = dataclasses.replace(
                            mb_sb[:],
                            ap=[mb_sb.ap[0], [128 * (2 - s0), 2], [0, 2],
                                [1, 128]],
                            offset=mb_sb.offset + 128 * s0)
                        nc.vector.tensor_mul(edges, edges, mop)
                        if qb == 0:
                            m1 = dataclasses.replace(
                                mb_sb[:, 3, :],
                                ap=[mb_sb.ap[0], [0, 2], [1, 128]])
                            nc.vector.tensor_mul(pt[:, 1, :, :],
                                                 pt[:, 1, :, :], m1)
                        if dbg is not None and qb == 3 and g == 1 and jh == 1:
                            nc.sync.dma_start(
                                out=dbg["d_pt"],
                                in_=pt[:].rearrange("p a b c -> p (a b c)"))
                        # att@v flipped: stat = pt block, mov = v (+ones col)
                        for j2 in range(2):
                            for cc in range(3):
                                nc.tensor.matmul(
                                    yu[:, 2 * jh + j2, 0:65],
                                    pt[:, cc, j2, :],
                                    v65[:, qb + cc, g, :],
                                    start=(cc == 0), stop=(cc == 2))
                    # denominators: column 64 -> reciprocal -> normalize
                    rsb = atts.tile([128, 4], F32, tag="rsb")
                    nc.vector.reciprocal(rsb[:], yu[:, :, 64])
                    yv = atts.tile([128, 4, 64], BF16, tag="yv")
                    rbc = dataclasses.replace(
                        rsb[:], ap=[rsb.ap[0], [1, 4], [0, 64]])
                    nc.vector.tensor_mul(yv[:], yu[:, :, 0:64], rbc)
                    if dbg is not None and qb == 3 and g == 1:
                        nc.sync.dma_start(out=dbg["d_rs"], in_=rsb[:])
                        nc.sync.dma_start(
                            out=dbg["d_yv"],
                            in_=yv[:].rearrange("p a b -> p (a b)"))
                    # transpose [q, d] -> [d, q] for the output projection
                    tp = yups.tile([128, 2, 128], BF16, tag="tp")
                    for jj in range(2):
                        nc.tensor.transpose(
                            tp[:, jj, :],
                            yv[:].rearrange("p a b -> p (a b)")[
                                :, jj * 128:(jj + 1) * 128],
                            idt_sb[:])
                    ydst = dataclasses.replace(
                        yT[:, 2 * g, qb * 128:(qb + 1) * 128],
                        ap=[yT.ap[0], [TL, 2], [1, 128]])
                    nc.vector.tensor_copy(ydst, tp[:])
                # output projection for this qb (t-tile == qb)
                o_sb = atts.tile([128, C], BF16, tag="o_sb")
                for oc in range(2):
                    op = ops.tile([128, 512], F32, tag="op")
                    for pr in range(8):
                        nc.tensor.matmul(
                            op[:],
                            yT[:, pr, qb * 128:(qb + 1) * 128],
                            wp_sb[:, pr, oc * 512:(oc + 1) * 512],
                            start=(pr == 0), stop=(pr == 7))
                    nc.vector.tensor_copy(o_sb[:, oc * 512:(oc + 1) * 512],
                                          op[:])
                nc.scalar.dma_start(out=out[qb * 128:(qb + 1) * 128, :],
                                    in_=o_sb[:])
            if dbg is not None:
                nc.sync.dma_start(out=dbg["d_yT"],
                                  in_=yT[:].rearrange("p a b -> p (a b)"))


_PROGRAM_CACHE = {}


def _get_program():
    if "nc" not in _PROGRAM_CACHE:
        _PROGRAM_CACHE["nc"] = _build_program()
    return _PROGRAM_CACHE["nc"]


def _hi_lo(a):
    hi = a.astype(F8)
    lo = (a - hi.astype(np.float32)).astype(F8)
    return hi, lo


def _interleave(a):
    """(1024, N) channel-major -> (128, 8, N) partition-major layout."""
    n = a.shape[1]
    return np.ascontiguousarray(a.reshape(8, 128, n).transpose(1, 0, 2))


def prepare_in_maps(x, freqs_cos, freqs_sin, w_attn, b_attn, w_proj, b_proj):
    x = np.asarray(x, dtype=np.float32)
    freqs_cos = np.asarray(freqs_cos, dtype=np.float32)
    freqs_sin = np.asarray(freqs_sin, dtype=np.float32)
    w_attn = np.asarray(w_attn, dtype=np.float32)
    b_attn = np.asarray(b_attn, dtype=np.float32)
    w_proj = np.asarray(w_proj, dtype=np.float32)
    assert not np.any(b_attn), "kernel assumes zero qkv bias"

    # q/k channel permutation: evens block then odds block; within each
    # 128-wide group block the order is (d-pair, head) so the rope output
    # repartitions to [32, 4, T] with a single DMA.
    q_ev, q_od = [], []
    for c4 in range(4):
        q_ev += [(4 * c4 + j) * HD + 2 * p for p in range(32) for j in range(4)]
        q_od += [(4 * c4 + j) * HD + 2 * p + 1 for p in range(32) for j in range(4)]
    k_ev = [H * HD + g * HD + 2 * p for p in range(32) for g in range(KV)]
    k_od = [H * HD + g * HD + 2 * p + 1 for p in range(32) for g in range(KV)]
    q_perm = np.array(q_ev + q_od)
    k_perm = np.array(k_ev + k_od)
    wqk = np.ascontiguousarray(
        w_attn[np.concatenate([q_perm, k_perm])].T) * WS     # (1024, 1280)
    wqk_h, wqk_l = _hi_lo(wqk)
    wv_f = np.ascontiguousarray(w_attn[(H + KV) * HD:].T) * WS
    wv_h, wv_l = _hi_lo(wv_f)
    wp_h = np.ascontiguousarray(w_proj.T).astype(BF)

    # row r of a projection tile is d-pair r//4 (d-pair-major order)
    cos4 = np.repeat(freqs_cos.T, 4, axis=0).astype(np.float32)   # (128, T)
    sin4 = np.repeat(freqs_sin.T, 4, axis=0).astype(np.float32)

    # multiplicative mask bank (0/1), slots:
    #   0: cc0 for qb<=1   1: cc0 standard   2: cc2   3: cc1 for qb0
    k = np.arange(128)[:, None]
    q = np.arange(128)[None, :]
    upper = (k > q).astype(np.float32)
    lower = (k <= q).astype(np.float32)

    def mask_bank(first_half):
        a = np.zeros((128, 128), np.float32) if first_half else upper
        d = np.zeros((128, 128), np.float32) if first_half else \
            np.ones((128, 128), np.float32)
        return np.stack([a, upper, lower, d], axis=1).astype(BF)

    idt = np.eye(128, dtype=np.float32).astype(BF)

    in_maps = []
    for core in range(8):
        b, h = divmod(core, 2)
        t0 = h * TL
        xs = np.zeros((TH, C), dtype=np.float32)
        lo = max(0, t0 - WIN)
        xs[TH - (t0 + TL - lo):] = x[b, lo:t0 + TL]
        xT = np.ascontiguousarray(xs.T)            # (1024, 1280)
        xT_h, xT_l = _hi_lo(xT)
        cpad = np.zeros((128, TH), dtype=np.float32)
        spad = np.zeros((128, TH), dtype=np.float32)
        cpad[:, TH - (t0 + TL - lo):] = cos4[:, lo:t0 + TL]
        spad[:, TH - (t0 + TL - lo):] = sin4[:, lo:t0 + TL]
        in_maps.append({
            "xh": _interleave(xT_h), "xl": _interleave(xT_l),
            "wqkh": _interleave(wqk_h), "wqkl": _interleave(wqk_l),
            "wvh": _interleave(wv_h), "wvl": _interleave(wv_l),
            "wp": _interleave(wp_h),
            "cq": np.ascontiguousarray(
                np.stack([cos4[:, t0:t0 + TL],
                          sin4[:, t0:t0 + TL]], axis=1)).astype(BF),
            "ck": np.ascontiguousarray(
                np.stack([cpad, spad], axis=1)).astype(BF),
            "mbm": mask_bank(h == 0), "idt": idt,
        })

    return in_maps


def kernel(**inputs):
    in_maps = prepare_in_maps(**inputs)
    nc = _get_program()
    res = run_bass_kernel_spmd(nc, in_maps, list(range(8)))
    return _gather(res, np.asarray(inputs["b_proj"], dtype=np.float32))


def _gather(res, b_proj):
    out = np.empty((B, T, C), dtype=np.float32)
    for core in range(8):
        b, h = divmod(core, 2)
        out[b, h * TL:(h + 1) * TL] = np.asarray(
            res.results[core]["out"], dtype=np.float32)
    if np.any(b_proj):
        out += b_proj
    return out
